# revision 4
# baseline (speedup 1.0000x reference)
"""Classical ray marcher (volume rendering) Bass kernel for 8 Trainium2 cores.

Problem: B=8, R=16384 rays, S=64 samples/ray, C=3 channels.
  dens   = softplus(densities)
  deltas = diff(depths) with last delta = 1e10
  alpha  = 1 - exp(-deltas*dens)
  t      = 1 - alpha + 1e-10
  trans  = cumprod(t) with leading 1          (per ray, over S)
  w      = alpha * trans[:-1]
  rgb    = sum_s w*colors ; depth = sum_s w*depths ; ft = trans[-1]

Sharding: core i handles batch i (embarrassingly parallel over rays).
On-chip layout ("layout A"): 128 rays on partitions, G rays x 64 samples
on the free dim. The per-ray cumprod runs as ONE hardware linear-recurrence
scan per tile (state = t*state + b), where b is zero except at each ray's
first sample (carries t[0]); that resets the recurrence at ray boundaries.
"""

import numpy as np

import concourse.bacc as bacc
import concourse.mybir as mybir
from concourse import tile
from concourse.bass_utils import run_bass_kernel_spmd

F32 = mybir.dt.float32
AF = mybir.ActivationFunctionType
OP = mybir.AluOpType

B, R, S, C = 8, 16384, 64, 3
P = 128           # SBUF partitions (rays per partition-tile)
EPS = 1e-10

N_CORES = 8

_BUILT = {}


def _build(n_rays: int, g: int):
    """Build the single-core Bass module for n_rays rays, g rays/partition/tile."""
    nc = bacc.Bacc("TRN2", target_bir_lowering=False, debug=False)

    w_free = g * S            # free width of S-sized tiles
    wc_free = g * S * C       # free width of color tiles
    rays_per_tile = P * g
    n_tiles = n_rays // rays_per_tile
    assert n_tiles * rays_per_tile == n_rays

    colors_d = nc.dram_tensor("colors", [n_rays, S * C], F32, kind="ExternalInput")
    dens_d = nc.dram_tensor("densities", [n_rays, S], F32, kind="ExternalInput")
    depths_d = nc.dram_tensor("depths", [n_rays, S], F32, kind="ExternalInput")

    w_out_d = nc.dram_tensor("weights", [n_rays, S], F32, kind="ExternalOutput")
    rgb_d = nc.dram_tensor("rgb", [n_rays, C], F32, kind="ExternalOutput")
    depth_d = nc.dram_tensor("depth_out", [n_rays], F32, kind="ExternalOutput")
    ft_d = nc.dram_tensor("ft", [n_rays], F32, kind="ExternalOutput")

    with tile.TileContext(nc) as tc:
        with (
            tc.tile_pool(name="const", bufs=1) as constp,
            tc.tile_pool(name="io", bufs=3) as io,
            tc.tile_pool(name="work", bufs=2) as work,
        ):
            # scan's additive input: all zeros except each ray's s=0 column,
            # which is rewritten per tile with that ray's t[0].
            b1 = constp.tile([P, w_free], F32)
            nc.vector.memset(b1[:], 0.0)

            for i in range(n_tiles):
                rays = slice(i * rays_per_tile, (i + 1) * rays_per_tile)

                col_t = io.tile([P, wc_free], F32, tag="col")
                dep_t = io.tile([P, w_free], F32, tag="dep")
                den_t = io.tile([P, w_free], F32, tag="den")
                nc.sync.dma_start(
                    col_t[:], colors_d.ap()[rays].rearrange("(p g) w -> p (g w)", p=P)
                )
                nc.sync.dma_start(
                    dep_t[:], depths_d.ap()[rays].rearrange("(p g) w -> p (g w)", p=P)
                )
                nc.sync.dma_start(
                    den_t[:], dens_d.ap()[rays].rearrange("(p g) w -> p (g w)", p=P)
                )

                dep3 = dep_t.rearrange("p (g s) -> p g s", g=g)
                den3 = den_t.rearrange("p (g s) -> p g s", g=g)

                # dens = softplus(raw) = Ln(Exp(raw)+1)  [ACT x2, same table]
                dex = work.tile([P, w_free], F32, tag="dex")
                nc.scalar.activation(dex[:], den_t[:], AF.Exp)
                dens = work.tile([P, w_free], F32, tag="dens")
                nc.scalar.activation(dens[:], dex[:], AF.Ln, bias=1.0)
                dens3 = dens.rearrange("p (g s) -> p g s", g=g)

                # delta[s] = depths[s+1]-depths[s], s<63 [DVE]
                delta = work.tile([P, w_free], F32, tag="delta")
                delta3 = delta.rearrange("p (g s) -> p g s", g=g)
                nc.vector.tensor_tensor(
                    delta3[:, :, 0:S - 1], dep3[:, :, 1:S], dep3[:, :, 0:S - 1],
                    OP.subtract,
                )

                # p = delta*dens (s<63)                  [DVE]
                pt = work.tile([P, w_free], F32, tag="pt")
                pt3 = pt.rearrange("p (g s) -> p g s", g=g)
                nc.vector.tensor_tensor(
                    pt3[:, :, 0:S - 1], delta3[:, :, 0:S - 1], dens3[:, :, 0:S - 1],
                    OP.mult,
                )

                # e = exp(-p) (s<63)                     [ACT]
                e = work.tile([P, w_free], F32, tag="e")
                e3 = e.rearrange("p (g s) -> p g s", g=g)
                nc.scalar.activation(
                    e3[:, :, 0:S - 1], pt3[:, :, 0:S - 1], AF.Exp, scale=-1.0
                )

                # t = e + EPS (s<63); t[63] = EPS exactly (delta=1e10 kills exp)
                t = work.tile([P, w_free], F32, tag="t")
                t3 = t.rearrange("p (g s) -> p g s", g=g)
                nc.scalar.activation(
                    t3[:, :, 0:S - 1], e3[:, :, 0:S - 1], AF.Copy, bias=EPS
                )
                nc.gpsimd.memset(t3[:, :, S - 1:S], EPS)

                # alpha = 1 - e (s<63); alpha[63] = 1
                alpha = work.tile([P, w_free], F32, tag="alpha")
                alpha3 = alpha.rearrange("p (g s) -> p g s", g=g)
                nc.scalar.activation(
                    alpha3[:, :, 0:S - 1], e3[:, :, 0:S - 1], AF.Copy,
                    bias=1.0, scale=-1.0,
                )
                nc.gpsimd.memset(alpha3[:, :, S - 1:S], 1.0)

                # segment-reset plumbing: b1[s=0] = t[0]; then t[0] := 0
                b13 = b1.rearrange("p (g s) -> p g s", g=g)
                nc.scalar.copy(b13[:, :, 0:1], t3[:, :, 0:1])
                nc.gpsimd.memset(t3[:, :, 0:1], 0.0)

                # c[s] = inclusive cumprod of t per ray   [DVE scan]
                # state = t[s]*state + b1[s]; at s=0: 0*state + t[0] = t[0]
                c = work.tile([P, w_free], F32, tag="c")
                nc.vector.tensor_tensor_scan(
                    c[:], t[:], b1[:], 0.0, OP.mult, OP.add
                )
                c3 = c.rearrange("p (g s) -> p g s", g=g)

                # w[0] = alpha[0]; w[s] = alpha[s] * c[s-1]
                w = work.tile([P, w_free], F32, tag="w")
                w3 = w.rearrange("p (g s) -> p g s", g=g)
                nc.scalar.copy(w3[:, :, 0:1], alpha3[:, :, 0:1])
                nc.vector.tensor_tensor(
                    w3[:, :, 1:S], alpha3[:, :, 1:S], c3[:, :, 0:S - 1], OP.mult
                )

                # wc = w (broadcast over C) * colors      [DVE]
                wc = work.tile([P, wc_free], F32, tag="wc")
                wc4 = wc.rearrange("p (g s ch) -> p g s ch", g=g, s=S)
                col4 = col_t.rearrange("p (g s ch) -> p g s ch", g=g, s=S)
                wbc = w3.unsqueeze(3).broadcast_to([P, g, S, C])
                nc.vector.tensor_tensor(wc4[:], col4[:], wbc, OP.mult)

                # rgb[g,c] = sum_s wc[g,s,c]              [DVE reduce over S]
                rgb_t = work.tile([P, g * C], F32, tag="rgb_t")
                rgb3 = rgb_t.rearrange("p (g ch) -> p g ch", g=g)
                nc.vector.tensor_reduce(
                    rgb3[:], wc4.transpose([0, 1, 3, 2]), mybir.AxisListType.X, OP.add
                )

                # wd = w * depths; depth = sum_s wd       [DVE]
                wd = work.tile([P, w_free], F32, tag="wd")
                nc.vector.tensor_tensor(wd[:], w[:], dep_t[:], OP.mult)
                dep_o = work.tile([P, g], F32, tag="dep_o")
                nc.vector.tensor_reduce(
                    dep_o[:], wd.rearrange("p (g s) -> p g s", g=g),
                    mybir.AxisListType.X, OP.add,
                )

                # ft = c[63] (full cumprod incl. the 1e-10 last factor)
                ft_t = work.tile([P, g], F32, tag="ft_t")
                nc.scalar.copy(ft_t[:].unsqueeze(2), c3[:, :, S - 1:S])

                # stores
                nc.sync.dma_start(
                    w_out_d.ap()[rays].rearrange("(p g) s -> p (g s)", p=P), w[:]
                )
                nc.sync.dma_start(
                    rgb_d.ap()[rays].rearrange("(p g) c -> p (g c)", p=P), rgb_t[:]
                )
                nc.sync.dma_start(
                    depth_d.ap()[rays].rearrange("(p g) -> p g", p=P), dep_o[:]
                )
                nc.sync.dma_start(
                    ft_d.ap()[rays].rearrange("(p g) -> p g", p=P), ft_t[:]
                )

    nc.compile()
    return nc


def _get_nc(n_rays=R, g=16):
    key = (n_rays, g)
    if key not in _BUILT:
        _BUILT[key] = _build(n_rays, g)
    return _BUILT[key]


def _run(in_maps, n_rays=R, g=16, trace=False, **kw):
    nc = _get_nc(n_rays, g)
    return run_bass_kernel_spmd(nc, in_maps, list(range(len(in_maps))), trace=trace, **kw)


def kernel(colors, densities, depths):
    """Full-input entry point: colors [8,16384,64,3], densities/depths [8,16384,64,1].

    Returns (rgb_final [B,R,C], depth [B,R,1], weights [B,R,S,1], final_trans [B,R]).
    """
    colors = np.ascontiguousarray(colors, dtype=np.float32)
    densities = np.ascontiguousarray(densities, dtype=np.float32)
    depths = np.ascontiguousarray(depths, dtype=np.float32)

    in_maps = [
        {
            "colors": colors[i].reshape(R, S * C),
            "densities": densities[i].reshape(R, S),
            "depths": depths[i].reshape(R, S),
        }
        for i in range(B)
    ]
    res = _run(in_maps).results

    rgb = np.stack([res[i]["rgb"] for i in range(B)])                    # [B,R,C]
    depth = np.stack([res[i]["depth_out"] for i in range(B)])[..., None]  # [B,R,1]
    weights = np.stack([res[i]["weights"] for i in range(B)])[..., None]  # [B,R,S,1]
    ft = np.stack([res[i]["ft"] for i in range(B)])                      # [B,R]
    return rgb, depth, weights, ft
